# revision 30
# baseline (speedup 1.0000x reference)
"""BiLevelRoutingAttention Trainium2 kernel.

Sharding: data-parallel over (T*B)=8 cores; core = b*4 + t.
Host: windowize + transpose + region-routing top-k (0.005% of FLOPs).
Device per core (one (t,b) shard, 64 windows x 128 tokens x 256 ch):
  stage 1: qkv projection via compensated f16-hi + fp8-lo
    (x@W = xh@Wh [f16, 2048x-scaled] + xh@Wl + xl@Wh [fp8 DoubleRow,
    sign/scale baked on host]; effective qkv error ~2^-15, enough for
    the chaotic spike thresholds where plain f16 fails); LIF bits. q
    is produced directly
    in transposed [channel, token] layout (W as the stationary operand)
    so no per-window transposes are needed; k/v bits land interleaved
    per window as [k 256 | v 256 | ones 8] in fp8e4 (bits are exact).
  stage 2 per window: gathered kv/ksum contraction as fp8 DoubleRow
    matmuls (2 k-tiles = 2 gathered windows per matmul, 2x PE rate,
    exact integer arithmetic), masked block-diagonal kv+ksum copy,
    linear attention numerator+denominator in one f16 matmul pair,
    fused reciprocal scale, f16 transpose + f16 output projection
    (f16 proj adds only ~3e-4 L2), output DMA batched 4 windows.
The top-k window indices (which depend only on batch b) are baked into
the program; cores select their variant via tc.If(partition_id).
"""

import numpy as np

# problem constants (hardcoded per contract)
T, B, Lt, Lh, Lw, C = 4, 2, 8, 32, 32, 256
WT, WH, WW = 4, 4, 4
NW = WT * WH * WW              # 64 windows
PT, PH, PW = Lt // WT, Lh // WH, Lw // WW
WS = PT * PH * PW              # 128 tokens per window
H, HD = 8, C // 8
TOPK = 4
NTOK = NW * WS                 # 8192 tokens per (t,b) shard
N_CORES = 8
WREC = 528                     # kv_bits record: k256|v256|ones8|pad8 (16B-aligned
                               # stride required by fp8 DoubleRow ldweights)

last_results = None            # stashed for test harness
last_nc = None
last_in_maps = None


def _windowize(x):
    xw = x.reshape(T, B, WT, PT, WH, PH, WW, PW, C)
    xw = xw.transpose(0, 1, 2, 4, 6, 3, 5, 7, 8).reshape(T, B, NW, WS, C)
    return xw


def _unwindowize(ow):
    o = ow.reshape(T, B, WT, WH, WW, PT, PH, PW, C)
    o = o.transpose(0, 1, 2, 5, 3, 6, 4, 7, 8).reshape(T, B, Lt, Lh, Lw, C)
    return o


def _routing_idx(xw32):
    """Mimic reference routing in fp32: region scores -> top-4 window idx."""
    region = xw32.sum(0).mean(2)                           # [B,NW,C]
    scores = np.einsum('bic,bjc->bij', region, region) * np.float32(HD ** -0.5)
    # jax.lax.top_k tie-break = lowest index first; stable argsort matches
    idx = np.argsort(-scores, axis=-1, kind='stable')[:, :, :TOPK]
    return idx                                             # [B,NW,TOPK]


def _build_program(idx_by_b, bq_zero=False, bp_zero=False,
                   profile_single=False):
    import concourse.bass as bass
    import concourse.mybir as mybir
    import concourse.tile as tile
    from concourse import bacc
    from concourse.masks import make_identity

    f32 = mybir.dt.float32
    f16 = mybir.dt.float16
    fp8 = mybir.dt.float8e4
    DR = mybir.MatmulPerfMode.DoubleRow

    nc = bacc.Bacc("TRN2", target_bir_lowering=False, debug=False,
                   num_devices=N_CORES)

    xwT = nc.dram_tensor("xwT", [C, NTOK], f16, kind="ExternalInput").ap()
    xw8 = nc.dram_tensor("xw8", [C, NTOK], fp8, kind="ExternalInput").ap()
    xl8d = nc.dram_tensor("xl8", [C, NTOK], fp8, kind="ExternalInput").ap()
    wq = nc.dram_tensor("wq", [C, 3 * C], f16, kind="ExternalInput").ap()
    wl8d = nc.dram_tensor("wl8", [C, 3 * C], fp8, kind="ExternalInput").ap()
    wh8d = nc.dram_tensor("wh8", [C, 3 * C], fp8, kind="ExternalInput").ap()
    bq = nc.dram_tensor("bq", [3 * C], f32, kind="ExternalInput").ap()
    wp = nc.dram_tensor("wp", [C, C], f16, kind="ExternalInput").ap()
    bp = nc.dram_tensor("bp", [C], f32, kind="ExternalInput").ap()
    masks = nc.dram_tensor("masks", [128, 528], f16, kind="ExternalInput").ap()
    out_d = nc.dram_tensor("out", [NTOK, C], f32, kind="ExternalOutput").ap()

    with tile.TileContext(nc) as tc:
        with (
            tc.tile_pool(name="const", bufs=1) as const_pool,
            tc.tile_pool(name="bits", bufs=1) as bits_pool,
            tc.tile_pool(name="xt", bufs=4) as xt_pool,
            tc.tile_pool(name="work", bufs=4) as work_pool,
            tc.tile_pool(name="ob", bufs=2) as ob_pool,
            tc.tile_pool(name="tpsum", bufs=2, space="PSUM") as tpsum,
        ):
            # ---- resident constants ----
            wq_sb = const_pool.tile([128, 2 * 768], f16, tag="wq")
            wl8_sb = const_pool.tile([128, 2 * 768], fp8, tag="wl8")
            wh8_sb = const_pool.tile([128, 2 * 768], fp8, tag="wh8")
            for kc in range(2):
                nc.sync.dma_start(wq_sb[:, kc * 768:(kc + 1) * 768],
                                  wq[kc * 128:(kc + 1) * 128, :])
                nc.sync.dma_start(wl8_sb[:, kc * 768:(kc + 1) * 768],
                                  wl8d[kc * 128:(kc + 1) * 128, :])
                nc.sync.dma_start(wh8_sb[:, kc * 768:(kc + 1) * 768],
                                  wh8d[kc * 128:(kc + 1) * 128, :])
            wp_sb = const_pool.tile([128, 2 * 256], f16, tag="wp")
            for kc in range(2):
                nc.sync.dma_start(wp_sb[:, kc * 256:(kc + 1) * 256],
                                  wp[kc * 128:(kc + 1) * 128, :])
            ident_h = const_pool.tile([128, 128], f16, tag="idh")
            make_identity(nc, ident_h)

            ones_row = const_pool.tile([1, 128], f32, tag="ones")
            nc.vector.memset(ones_row, 1.0)
            bq_row = const_pool.tile([1, 768], f32, tag="bqr")
            nc.sync.dma_start(bq_row, bq[None, :])
            bp_row = const_pool.tile([1, 256], f32, tag="bpr")
            nc.sync.dma_start(bp_row, bp[None, :])
            mask_sb = const_pool.tile([128, 528], f16, tag="masks")
            nc.sync.dma_start(mask_sb, masks)

            # thresholds: spike(x) fires iff qkv + b >= 2
            thr_kv = const_pool.tile([128, 512], f32, tag="thrkv")
            thr_q = const_pool.tile([128, 2], f32, tag="thrq")
            bq_qT = const_pool.tile([128, 2], f32, tag="bqqt")
            nc.sync.dma_start(bq_qT, bq[0:256].rearrange("(c p) -> p c", p=128))
            # thresholds carry the 2048x hi-term scaling (exact f32)
            nc.vector.tensor_scalar(out=thr_q, in0=bq_qT,
                                    scalar1=-2048.0, scalar2=4096.0,
                                    op0=mybir.AluOpType.mult,
                                    op1=mybir.AluOpType.add)
            bp_bc2 = const_pool.tile([128, 512], f32, tag="bpbc")

            # ---- bit tensors (resident) ----
            # q bits transposed: [qc-in-chunk, tok], chunk hq at col hq*NTOK
            qT_bits = bits_pool.tile([128, 2 * NTOK], f16, tag="qb")
            # k/v bits per window: k 0:256 | v 256:512 | ones 512:520
            kv_bits = bits_pool.tile([128, NW * WREC], fp8, tag="kvb")
            kv_r = kv_bits.rearrange("p (w d) -> p w d", d=WREC)
            nc.vector.memset(kv_r[:, :, 512:520], 1.0)

            # ---- stage 1: qkv projection + LIF bits ----
            with tc.tile_pool(name="s1_ps", bufs=3, space="PSUM") as s1_psum:
                # broadcast rows via ones-column matmul
                bc_ps = s1_psum.tile([128, 512], f32, tag="s1")
                nc.tensor.matmul(bc_ps, ones_row, bq_row[:, 256:768],
                                 start=True, stop=True)
                nc.vector.tensor_scalar(out=thr_kv, in0=bc_ps,
                                        scalar1=-2048.0, scalar2=4096.0,
                                        op0=mybir.AluOpType.mult,
                                        op1=mybir.AluOpType.add)
                bc_ps2 = s1_psum.tile([128, 512], f32, tag="s1")
                nc.tensor.matmul(bc_ps2[:, 0:256], ones_row, bp_row,
                                 start=True, stop=True)
                nc.tensor.matmul(bc_ps2[:, 256:512], ones_row, bp_row,
                                 start=True, stop=True)
                nc.scalar.copy(bp_bc2, bc_ps2)

                for bi in range(8):                        # 1024-token blocks
                    bs = slice(bi * 1024, (bi + 1) * 1024)
                    xt0 = xt_pool.tile([128, 1024], f16, tag="xt")
                    xt1 = xt_pool.tile([128, 1024], f16, tag="xt")
                    x8 = xt_pool.tile([128, 2 * 1024], fp8, tag="x8")
                    l8 = xt_pool.tile([128, 2 * 1024], fp8, tag="l8")
                    nc.sync.dma_start(xt0, xwT[0:128, bs])
                    nc.sync.dma_start(xt1, xwT[128:256, bs])
                    nc.sync.dma_start(x8[:, 0:1024], xw8[0:128, bs])
                    nc.sync.dma_start(x8[:, 1024:2048], xw8[128:256, bs])
                    nc.sync.dma_start(l8[:, 0:1024], xl8d[0:128, bs])
                    nc.sync.dma_start(l8[:, 1024:2048], xl8d[128:256, bs])
                    x8r = x8.rearrange("p (t n) -> p t n", t=2)
                    l8r = l8.rearrange("p (t n) -> p t n", t=2)
                    # q part: [qc, tok] = wq_q.T @ xT, 512-token sub-blocks
                    for sb in range(2):
                        tcol = bi * 1024 + sb * 512
                        for hq in range(2):
                            ps = s1_psum.tile([128, 512], f32, tag="s1")
                            pl = s1_psum.tile([128, 512], f32, tag="s1lo")
                            ss = slice(sb * 512, (sb + 1) * 512)
                            wsl = slice(hq * 128, (hq + 1) * 128)
                            nc.tensor.matmul(ps, wq_sb[:, wsl], xt0[:, ss],
                                             start=True, stop=False)
                            nc.tensor.matmul(
                                ps, wq_sb[:, 768 + hq * 128:768 + (hq + 1) * 128],
                                xt1[:, ss], start=False, stop=True)
                            nc.tensor.matmul(
                                pl,
                                wl8_sb.rearrange("p (t n) -> p t n", t=2)[:, :, wsl],
                                x8r[:, :, ss],
                                start=True, stop=False, perf_mode=DR)
                            nc.tensor.matmul(
                                pl,
                                wh8_sb.rearrange("p (t n) -> p t n", t=2)[:, :, wsl],
                                l8r[:, :, ss],
                                start=False, stop=True, perf_mode=DR)
                            pl_sb = work_pool.tile([128, 512], f32,
                                                   tag="s1t")
                            qdst = qT_bits[:, hq * NTOK + tcol:
                                           hq * NTOK + tcol + 512]
                            if bq_zero:
                                # drain absorbs thr: pl_sb = pl + 4096;
                                # bits = ps >= pl_sb
                                nc.scalar.activation(
                                    pl_sb, pl,
                                    mybir.ActivationFunctionType.Copy,
                                    bias=4096.0)
                                nc.vector.tensor_tensor(
                                    out=qdst, in0=ps, in1=pl_sb,
                                    op=mybir.AluOpType.is_ge)
                            else:
                                nc.scalar.copy(pl_sb, pl)
                                # bits = (ps_hi - thr2048) >= ps_lo'
                                nc.vector.scalar_tensor_tensor(
                                    out=qdst, in0=ps,
                                    scalar=thr_q[:, hq:hq + 1],
                                    in1=pl_sb,
                                    op0=mybir.AluOpType.subtract,
                                    op1=mybir.AluOpType.is_ge)
                    # k/v part: [tok, c] per window
                    for wi in range(8):
                        n = bi * 8 + wi
                        ps = s1_psum.tile([128, 512], f32, tag="s1")
                        pl = s1_psum.tile([128, 512], f32, tag="s1lo")
                        ws_ = slice(wi * 128, (wi + 1) * 128)
                        nc.tensor.matmul(ps, xt0[:, ws_], wq_sb[:, 256:768],
                                         start=True, stop=False)
                        nc.tensor.matmul(ps, xt1[:, ws_],
                                         wq_sb[:, 768 + 256:768 + 768],
                                         start=False, stop=True)
                        wl8_r = wl8_sb.rearrange("p (t n) -> p t n", t=2)
                        wh8_r = wh8_sb.rearrange("p (t n) -> p t n", t=2)
                        nc.tensor.matmul(pl, x8r[:, :, ws_],
                                         wl8_r[:, :, 256:768],
                                         start=True, stop=False, perf_mode=DR)
                        nc.tensor.matmul(pl, l8r[:, :, ws_],
                                         wh8_r[:, :, 256:768],
                                         start=False, stop=True, perf_mode=DR)
                        # qkv*2048 = ps_hi - ps_lo'; compare vs thr2048 (f32)
                        pl_sb = work_pool.tile([128, 512], f32, tag="s1t")
                        if bq_zero:
                            # drain absorbs thr; DVE: plain is_ge
                            nc.scalar.activation(
                                pl_sb, pl,
                                mybir.ActivationFunctionType.Copy,
                                bias=4096.0)
                            nc.vector.tensor_tensor(
                                out=kv_r[:, n, 0:512], in0=ps, in1=pl_sb,
                                op=mybir.AluOpType.is_ge)
                        else:
                            nc.scalar.copy(pl_sb, pl)
                            tmp = work_pool.tile([128, 512], f32, tag="s1u")
                            nc.vector.scalar_tensor_tensor(
                                out=tmp, in0=pl_sb, scalar=-1.0, in1=ps,
                                op0=mybir.AluOpType.mult,
                                op1=mybir.AluOpType.add)
                            nc.vector.tensor_tensor(
                                out=kv_r[:, n, 0:512], in0=tmp, in1=thr_kv,
                                op=mybir.AluOpType.is_ge)

            # ---- stage 2: routed attention + projection ----
            def attention_stage(idx):
                with (
                    tc.tile_pool(name="kv_ps", bufs=3, space="PSUM") as kv_psum,
                    tc.tile_pool(name="at_ps", bufs=2, space="PSUM") as at_psum,
                    tc.tile_pool(name="pj_ps", bufs=1, space="PSUM") as pj_psum,
                ):
                    ob = None
                    for n in range(NW):
                        js = sorted(int(j) for j in idx[n])
                        pairs = [(js[0], js[1]), (js[2], js[3])]
                        kvs = work_pool.tile([128, 528], f16, tag="kvs")
                        kvf = work_pool.tile([128, 264], f16, tag="kvf")
                        for hf in range(2):
                            kvp = kv_psum.tile([128, 264], f32, tag="kv")
                            for pi, (ja, jb) in enumerate(pairs):
                                st = jb - ja
                                lhsT = kv_r[:, ja:jb + 1:st,
                                            hf * 128:(hf + 1) * 128]
                                rhs = kv_r[:, ja:jb + 1:st, 256:520]
                                nc.tensor.matmul(kvp, lhsT, rhs,
                                                 start=(pi == 0),
                                                 stop=(pi == 1),
                                                 perf_mode=DR)
                            if hf == 0:
                                # short path: DVE masks straight from PSUM
                                nc.vector.tensor_tensor(
                                    out=kvs[:, 0:264], in0=kvp,
                                    in1=mask_sb[:, 0:264],
                                    op=mybir.AluOpType.mult)
                            else:
                                # offload: ACT drains PSUM, Pool masks
                                nc.scalar.copy(kvf, kvp)
                                nc.gpsimd.tensor_tensor(
                                    out=kvs[:, 264:528], in0=kvf,
                                    in1=mask_sb[:, 264:528],
                                    op=mybir.AluOpType.mult)
                        # numerator + per-head D in one K=128 pair
                        ap_ = at_psum.tile([128, 264], f32, tag="at")
                        for hf in range(2):
                            nc.tensor.matmul(
                                ap_,
                                qT_bits[:, hf * NTOK + n * 128:
                                        hf * NTOK + (n + 1) * 128],
                                kvs[:, hf * 264:(hf + 1) * 264],
                                start=(hf == 0), stop=(hf == 1))
                        dr = work_pool.tile([128, 8], f32, tag="dr")
                        nc.vector.tensor_scalar_add(dr, ap_[:, 256:264], 1e-6)
                        nc.vector.reciprocal(dr, dr)
                        at = work_pool.tile([128, 256], f16, tag="attn")
                        nc.vector.tensor_tensor(
                            out=at.rearrange("p (h e) -> p h e", e=32),
                            in0=ap_[:, 0:256].rearrange("p (h e) -> p h e", e=32),
                            in1=dr.rearrange("p (h u) -> p h u", u=1)
                                 .to_broadcast([128, 8, 32]),
                            op=mybir.AluOpType.mult)
                        aT = work_pool.tile([128, 256], f16, tag="aT")
                        tp = tpsum.tile([128, 256], f16, tag="tr")
                        for kd in range(2):
                            nc.tensor.transpose(
                                tp[:, kd * 128:(kd + 1) * 128],
                                at[:, kd * 128:(kd + 1) * 128], ident_h)
                        nc.scalar.copy(aT, tp)
                        w4 = n % 4
                        if w4 % 2 == 0:
                            pp = pj_psum.tile([128, 512], f32, tag="pj")
                        pc = (w4 % 2) * 256
                        nc.tensor.matmul(pp[:, pc:pc + 256], aT[:, 0:128],
                                         wp_sb[:, 0:256],
                                         start=True, stop=False)
                        nc.tensor.matmul(pp[:, pc:pc + 256], aT[:, 128:256],
                                         wp_sb[:, 256:512],
                                         start=False, stop=True)
                        if w4 == 0:
                            ob = ob_pool.tile([128, 4 * 256], f32, tag="ob")
                        if w4 % 2 == 1:
                            obs = ob[:, (w4 - 1) * 256:(w4 + 1) * 256]
                            if bp_zero:
                                # zero bias: PSUM drain is a plain ACT copy
                                nc.scalar.copy(obs, pp)
                            else:
                                nc.vector.tensor_tensor(
                                    out=obs, in0=pp, in1=bp_bc2,
                                    op=mybir.AluOpType.add)
                        if w4 == 3:
                            n0 = n - 3
                            dst = out_d[n0 * 128:(n0 + 4) * 128, :] \
                                .rearrange("(w p) c -> p w c", p=128)
                            nc.sync.dma_start(
                                dst, ob.rearrange("p (w c) -> p w c", c=256))

            if profile_single:
                attention_stage(idx_by_b[0])
            else:
                pid = nc.partition_id()
                with tc.If(pid <= 3) as cmp:
                    attention_stage(idx_by_b[0])
                with cmp.Else():
                    attention_stage(idx_by_b[1])

    nc.compile()
    return nc


def kernel(x, W_qkv, b_qkv, W_proj, b_proj):
    global last_results, last_nc, last_in_maps
    from concourse import bass_utils

    x = np.asarray(x, dtype=np.float32)
    xw = _windowize(x)                                     # [T,B,NW,WS,C]
    idx = _routing_idx(xw)                                 # [B,NW,TOPK]

    bq_zero = bool(np.all(np.asarray(b_qkv) == 0.0))
    bp_zero = bool(np.all(np.asarray(b_proj) == 0.0))
    nc = _build_program(idx, bq_zero=bq_zero, bp_zero=bp_zero)

    mask = np.zeros((128, 528), np.float32)
    for hf in range(2):
        for cr in range(128):
            h = hf * 4 + cr // 32                  # global head of row cr
            mask[cr, hf * 264 + h * 32:hf * 264 + (h + 1) * 32] = 1.0
            mask[cr, hf * 264 + 256 + h] = 1.0

    import ml_dtypes
    E4 = ml_dtypes.float8_e4m3
    wq32 = np.asarray(W_qkv, np.float32)
    wqh = wq32.astype(np.float16)
    wl8 = (-2048.0 * (wq32 - wqh.astype(np.float32))).astype(E4)
    wh8 = (-wqh.astype(np.float32)).astype(E4)
    in_maps = []
    for core in range(N_CORES):
        b, t = divmod(core, T)
        xwT_c = np.ascontiguousarray(xw[t, b].reshape(NTOK, C).T)  # [C,NTOK]
        xh = xwT_c.astype(np.float16)
        xh_s = (xh.astype(np.float32) * 2048.0).astype(np.float16)
        xw8_ = xh.astype(np.float32).astype(E4)
        xl8_ = (2048.0 * (xwT_c - xh.astype(np.float32))).astype(E4)
        in_maps.append({
            "xwT": xh_s,
            "xw8": xw8_,
            "xl8": xl8_,
            "masks": mask.astype(np.float16),
            "wq": wqh,
            "wl8": wl8,
            "wh8": wh8,
            "bq": np.asarray(b_qkv, np.float32),
            "wp": np.asarray(W_proj, np.float16),
            "bp": np.asarray(b_proj, np.float32),
        })

    res = bass_utils.run_bass_kernel_spmd(
        nc, in_maps, core_ids=list(range(N_CORES)), trace=False)
    last_results = res
    last_nc, last_in_maps = nc, in_maps

    ow = np.empty((T, B, NW, WS, C), np.float32)
    for core in range(N_CORES):
        b, t = divmod(core, T)
        ow[t, b] = res.results[core]["out"].reshape(NW, WS, C)
    return _unwindowize(ow)


# revision 34
# speedup vs baseline: 1.2318x; 1.2318x over previous
"""BiLevelRoutingAttention Trainium2 kernel.

Sharding: data-parallel over (T*B)=8 cores; core = b*4 + t.
Host: windowize + transpose + region-routing top-k (0.005% of FLOPs).
Device per core (one (t,b) shard, 64 windows x 128 tokens x 256 ch):
  stage 1: qkv projection via compensated f16-hi + fp8-lo
    (x@W = xh@Wh [f16, 2048x-scaled] + xh@Wl + xl@Wh [fp8 DoubleRow,
    sign/scale baked on host]; effective qkv error ~2^-15, enough for
    the chaotic spike thresholds where plain f16 fails); LIF bits. q
    is produced directly
    in transposed [channel, token] layout (W as the stationary operand)
    so no per-window transposes are needed; k/v bits land interleaved
    per window as [k 256 | v 256 | ones 8] in fp8e4 (bits are exact).
  stage 2 per window: gathered kv/ksum contraction as fp8 DoubleRow
    matmuls (2 k-tiles = 2 gathered windows per matmul, 2x PE rate,
    exact integer arithmetic), masked block-diagonal kv+ksum copy,
    linear attention numerator+denominator in one f16 matmul pair,
    fused reciprocal scale, f16 transpose + f16 output projection
    (f16 proj adds only ~3e-4 L2), output DMA batched 4 windows.
The top-k window indices (which depend only on batch b) are baked into
the program; cores select their variant via tc.If(partition_id).
"""

import numpy as np

# problem constants (hardcoded per contract)
T, B, Lt, Lh, Lw, C = 4, 2, 8, 32, 32, 256
WT, WH, WW = 4, 4, 4
NW = WT * WH * WW              # 64 windows
PT, PH, PW = Lt // WT, Lh // WH, Lw // WW
WS = PT * PH * PW              # 128 tokens per window
H, HD = 8, C // 8
TOPK = 4
NTOK = NW * WS                 # 8192 tokens per (t,b) shard
N_CORES = 8
WREC = 528                     # kv_bits record: k256|v256|ones8|pad8 (16B-aligned
                               # stride required by fp8 DoubleRow ldweights)

last_results = None            # stashed for test harness
last_nc = None
last_in_maps = None


def _windowize(x):
    xw = x.reshape(T, B, WT, PT, WH, PH, WW, PW, C)
    xw = xw.transpose(0, 1, 2, 4, 6, 3, 5, 7, 8).reshape(T, B, NW, WS, C)
    return xw


def _unwindowize(ow):
    o = ow.reshape(T, B, WT, WH, WW, PT, PH, PW, C)
    o = o.transpose(0, 1, 2, 5, 3, 6, 4, 7, 8).reshape(T, B, Lt, Lh, Lw, C)
    return o


def _routing_idx(xw32):
    """Mimic reference routing in fp32: region scores -> top-4 window idx."""
    region = xw32.sum(0).mean(2)                           # [B,NW,C]
    scores = np.einsum('bic,bjc->bij', region, region) * np.float32(HD ** -0.5)
    # jax.lax.top_k tie-break = lowest index first; stable argsort matches
    idx = np.argsort(-scores, axis=-1, kind='stable')[:, :, :TOPK]
    return idx                                             # [B,NW,TOPK]


def _build_program(idx_by_b, bq_zero=False, bp_zero=False,
                   profile_single=False):
    import concourse.bass as bass
    import concourse.mybir as mybir
    import concourse.tile as tile
    from concourse import bacc
    from concourse.masks import make_identity

    f32 = mybir.dt.float32
    f16 = mybir.dt.float16
    fp8 = mybir.dt.float8e4
    DR = mybir.MatmulPerfMode.DoubleRow

    nc = bacc.Bacc("TRN2", target_bir_lowering=False, debug=False,
                   num_devices=N_CORES)

    xwT = nc.dram_tensor("xwT", [C, NTOK], f16, kind="ExternalInput").ap()
    xw8 = nc.dram_tensor("xw8", [C, NTOK], fp8, kind="ExternalInput").ap()
    xl8d = nc.dram_tensor("xl8", [C, NTOK], fp8, kind="ExternalInput").ap()
    wq = nc.dram_tensor("wq", [C, 3 * C], f16, kind="ExternalInput").ap()
    wl8d = nc.dram_tensor("wl8", [C, 3 * C], fp8, kind="ExternalInput").ap()
    wh8d = nc.dram_tensor("wh8", [C, 3 * C], fp8, kind="ExternalInput").ap()
    bq = nc.dram_tensor("bq", [3 * C], f32, kind="ExternalInput").ap()
    wp = nc.dram_tensor("wp", [C, C], f16, kind="ExternalInput").ap()
    bp = nc.dram_tensor("bp", [C], f32, kind="ExternalInput").ap()
    masks = nc.dram_tensor("masks", [128, 528], f16, kind="ExternalInput").ap()
    out_d = nc.dram_tensor("out", [NTOK, C], f32, kind="ExternalOutput").ap()

    with tile.TileContext(nc) as tc:
        with (
            tc.tile_pool(name="const", bufs=1) as const_pool,
            tc.tile_pool(name="bits", bufs=1) as bits_pool,
            tc.tile_pool(name="xt", bufs=4) as xt_pool,
            tc.tile_pool(name="work", bufs=4) as work_pool,
            tc.tile_pool(name="ob", bufs=2) as ob_pool,
            tc.tile_pool(name="tpsum", bufs=2, space="PSUM") as tpsum,
        ):
            # ---- resident constants ----
            wq_sb = const_pool.tile([128, 2 * 768], f16, tag="wq")
            wl8_sb = const_pool.tile([128, 2 * 768], fp8, tag="wl8")
            wh8_sb = const_pool.tile([128, 2 * 768], fp8, tag="wh8")
            for kc in range(2):
                nc.sync.dma_start(wq_sb[:, kc * 768:(kc + 1) * 768],
                                  wq[kc * 128:(kc + 1) * 128, :])
                nc.sync.dma_start(wl8_sb[:, kc * 768:(kc + 1) * 768],
                                  wl8d[kc * 128:(kc + 1) * 128, :])
                nc.sync.dma_start(wh8_sb[:, kc * 768:(kc + 1) * 768],
                                  wh8d[kc * 128:(kc + 1) * 128, :])
            wp_sb = const_pool.tile([128, 2 * 256], f16, tag="wp")
            mask_sb = const_pool.tile([128, 528], f16, tag="masks")
            ident_h = const_pool.tile([128, 128], f16, tag="idh")
            make_identity(nc, ident_h)

            if not (bq_zero and bp_zero):
                ones_row = const_pool.tile([1, 128], f32, tag="ones")
                nc.vector.memset(ones_row, 1.0)
            if not bq_zero:
                bq_row = const_pool.tile([1, 768], f32, tag="bqr")
                nc.sync.dma_start(bq_row, bq[None, :])
                # thresholds carry the 2048x hi-term scaling (exact f32)
                thr_kv = const_pool.tile([128, 512], f32, tag="thrkv")
                thr_q = const_pool.tile([128, 2], f32, tag="thrq")
                bq_qT = const_pool.tile([128, 2], f32, tag="bqqt")
                nc.sync.dma_start(bq_qT,
                                  bq[0:256].rearrange("(c p) -> p c", p=128))
                nc.vector.tensor_scalar(out=thr_q, in0=bq_qT,
                                        scalar1=-2048.0, scalar2=4096.0,
                                        op0=mybir.AluOpType.mult,
                                        op1=mybir.AluOpType.add)
            if not bp_zero:
                bp_row = const_pool.tile([1, 256], f32, tag="bpr")
                nc.sync.dma_start(bp_row, bp[None, :])
                bp_bc2 = const_pool.tile([128, 512], f32, tag="bpbc")

            # ---- bit tensors (resident) ----
            # q bits transposed: [qc-in-chunk, tok], chunk hq at col hq*NTOK
            qT_bits = bits_pool.tile([128, 2 * NTOK], f16, tag="qb")
            # k/v bits per window: k 0:256 | v 256:512 | ones 512:520
            kv_bits = bits_pool.tile([128, NW * WREC], fp8, tag="kvb")
            kv_r = kv_bits.rearrange("p (w d) -> p w d", d=WREC)
            nc.vector.memset(kv_r[:, :, 512:520], 1.0)

            # ---- stage 1: qkv projection + LIF bits ----
            with tc.tile_pool(name="s1_ps", bufs=3, space="PSUM") as s1_psum:
                # broadcast rows via ones-column matmul (general-bias paths)
                if not bq_zero:
                    bc_ps = s1_psum.tile([128, 512], f32, tag="s1")
                    nc.tensor.matmul(bc_ps, ones_row, bq_row[:, 256:768],
                                     start=True, stop=True)
                    nc.vector.tensor_scalar(out=thr_kv, in0=bc_ps,
                                            scalar1=-2048.0, scalar2=4096.0,
                                            op0=mybir.AluOpType.mult,
                                            op1=mybir.AluOpType.add)
                if not bp_zero:
                    bc_ps2 = s1_psum.tile([128, 512], f32, tag="s1")
                    nc.tensor.matmul(bc_ps2[:, 0:256], ones_row, bp_row,
                                     start=True, stop=True)
                    nc.tensor.matmul(bc_ps2[:, 256:512], ones_row, bp_row,
                                     start=True, stop=True)
                    nc.scalar.copy(bp_bc2, bc_ps2)

                for bi in range(8):                        # 1024-token blocks
                    bs = slice(bi * 1024, (bi + 1) * 1024)
                    xt0 = xt_pool.tile([128, 1024], f16, tag="xt")
                    xt1 = xt_pool.tile([128, 1024], f16, tag="xt")
                    x8 = xt_pool.tile([128, 2 * 1024], fp8, tag="x8")
                    l8 = xt_pool.tile([128, 2 * 1024], fp8, tag="l8")
                    nc.sync.dma_start(xt0, xwT[0:128, bs])
                    nc.sync.dma_start(xt1, xwT[128:256, bs])
                    nc.sync.dma_start(x8[:, 0:1024], xw8[0:128, bs])
                    nc.sync.dma_start(x8[:, 1024:2048], xw8[128:256, bs])
                    nc.sync.dma_start(l8[:, 0:1024], xl8d[0:128, bs])
                    nc.sync.dma_start(l8[:, 1024:2048], xl8d[128:256, bs])
                    x8r = x8.rearrange("p (t n) -> p t n", t=2)
                    l8r = l8.rearrange("p (t n) -> p t n", t=2)
                    # q part: [qc, tok] = wq_q.T @ xT, 512-token sub-blocks
                    for sb in range(2):
                        tcol = bi * 1024 + sb * 512
                        for hq in range(2):
                            ps = s1_psum.tile([128, 512], f32, tag="s1")
                            pl = s1_psum.tile([128, 512], f32, tag="s1lo")
                            ss = slice(sb * 512, (sb + 1) * 512)
                            wsl = slice(hq * 128, (hq + 1) * 128)
                            nc.tensor.matmul(ps, wq_sb[:, wsl], xt0[:, ss],
                                             start=True, stop=False)
                            nc.tensor.matmul(
                                ps, wq_sb[:, 768 + hq * 128:768 + (hq + 1) * 128],
                                xt1[:, ss], start=False, stop=True)
                            nc.tensor.matmul(
                                pl,
                                wl8_sb.rearrange("p (t n) -> p t n", t=2)[:, :, wsl],
                                x8r[:, :, ss],
                                start=True, stop=False, perf_mode=DR)
                            nc.tensor.matmul(
                                pl,
                                wh8_sb.rearrange("p (t n) -> p t n", t=2)[:, :, wsl],
                                l8r[:, :, ss],
                                start=False, stop=True, perf_mode=DR)
                            pl_sb = work_pool.tile([128, 512], f32,
                                                   tag="s1t")
                            qdst = qT_bits[:, hq * NTOK + tcol:
                                           hq * NTOK + tcol + 512]
                            if bq_zero:
                                # drain absorbs thr: pl_sb = pl + 4096;
                                # bits = ps >= pl_sb
                                nc.scalar.activation(
                                    pl_sb, pl,
                                    mybir.ActivationFunctionType.Copy,
                                    bias=4096.0)
                                nc.vector.tensor_tensor(
                                    out=qdst, in0=ps, in1=pl_sb,
                                    op=mybir.AluOpType.is_ge)
                            else:
                                nc.scalar.copy(pl_sb, pl)
                                # bits = (ps_hi - thr2048) >= ps_lo'
                                nc.vector.scalar_tensor_tensor(
                                    out=qdst, in0=ps,
                                    scalar=thr_q[:, hq:hq + 1],
                                    in1=pl_sb,
                                    op0=mybir.AluOpType.subtract,
                                    op1=mybir.AluOpType.is_ge)
                    # k/v part: [tok, c] per window
                    for wi in range(8):
                        n = bi * 8 + wi
                        ps = s1_psum.tile([128, 512], f32, tag="s1")
                        pl = s1_psum.tile([128, 512], f32, tag="s1lo")
                        ws_ = slice(wi * 128, (wi + 1) * 128)
                        nc.tensor.matmul(ps, xt0[:, ws_], wq_sb[:, 256:768],
                                         start=True, stop=False)
                        nc.tensor.matmul(ps, xt1[:, ws_],
                                         wq_sb[:, 768 + 256:768 + 768],
                                         start=False, stop=True)
                        wl8_r = wl8_sb.rearrange("p (t n) -> p t n", t=2)
                        wh8_r = wh8_sb.rearrange("p (t n) -> p t n", t=2)
                        nc.tensor.matmul(pl, x8r[:, :, ws_],
                                         wl8_r[:, :, 256:768],
                                         start=True, stop=False, perf_mode=DR)
                        nc.tensor.matmul(pl, l8r[:, :, ws_],
                                         wh8_r[:, :, 256:768],
                                         start=False, stop=True, perf_mode=DR)
                        # qkv*2048 = ps_hi - ps_lo'; compare vs thr2048 (f32)
                        pl_sb = work_pool.tile([128, 512], f32, tag="s1t")
                        if bq_zero:
                            # drain absorbs thr; DVE: plain is_ge
                            nc.scalar.activation(
                                pl_sb, pl,
                                mybir.ActivationFunctionType.Copy,
                                bias=4096.0)
                            nc.vector.tensor_tensor(
                                out=kv_r[:, n, 0:512], in0=ps, in1=pl_sb,
                                op=mybir.AluOpType.is_ge)
                        else:
                            nc.scalar.copy(pl_sb, pl)
                            tmp = work_pool.tile([128, 512], f32, tag="s1u")
                            nc.vector.scalar_tensor_tensor(
                                out=tmp, in0=pl_sb, scalar=-1.0, in1=ps,
                                op0=mybir.AluOpType.mult,
                                op1=mybir.AluOpType.add)
                            nc.vector.tensor_tensor(
                                out=kv_r[:, n, 0:512], in0=tmp, in1=thr_kv,
                                op=mybir.AluOpType.is_ge)

            # stage-2-only constants: DMA'd after stage 1 so they don't
            # delay the first x-tile loads at startup
            for kc in range(2):
                nc.sync.dma_start(wp_sb[:, kc * 256:(kc + 1) * 256],
                                  wp[kc * 128:(kc + 1) * 128, :])
            nc.sync.dma_start(mask_sb, masks)

            # ---- stage 2: routed attention + projection ----
            def attention_stage(idx):
                with (
                    tc.tile_pool(name="kv_ps", bufs=3, space="PSUM") as kv_psum,
                    tc.tile_pool(name="at_ps", bufs=2, space="PSUM") as at_psum,
                    tc.tile_pool(name="pj_ps", bufs=1, space="PSUM") as pj_psum,
                ):
                    ob = None
                    for n in range(NW):
                        js = sorted(int(j) for j in idx[n])
                        pairs = [(js[0], js[1]), (js[2], js[3])]
                        kvs = work_pool.tile([128, 528], f16, tag="kvs")
                        kvf = work_pool.tile([128, 264], f16, tag="kvf")
                        for hf in range(2):
                            kvp = kv_psum.tile([128, 264], f32, tag="kv")
                            for pi, (ja, jb) in enumerate(pairs):
                                st = jb - ja
                                lhsT = kv_r[:, ja:jb + 1:st,
                                            hf * 128:(hf + 1) * 128]
                                rhs = kv_r[:, ja:jb + 1:st, 256:520]
                                nc.tensor.matmul(kvp, lhsT, rhs,
                                                 start=(pi == 0),
                                                 stop=(pi == 1),
                                                 perf_mode=DR)
                            if hf == 0:
                                # short path: DVE masks straight from PSUM
                                nc.vector.tensor_tensor(
                                    out=kvs[:, 0:264], in0=kvp,
                                    in1=mask_sb[:, 0:264],
                                    op=mybir.AluOpType.mult)
                            else:
                                # offload: ACT drains PSUM, Pool masks
                                nc.scalar.copy(kvf, kvp)
                                nc.gpsimd.tensor_tensor(
                                    out=kvs[:, 264:528], in0=kvf,
                                    in1=mask_sb[:, 264:528],
                                    op=mybir.AluOpType.mult)
                        # numerator + per-head D in one K=128 pair
                        ap_ = at_psum.tile([128, 264], f32, tag="at")
                        for hf in range(2):
                            nc.tensor.matmul(
                                ap_,
                                qT_bits[:, hf * NTOK + n * 128:
                                        hf * NTOK + (n + 1) * 128],
                                kvs[:, hf * 264:(hf + 1) * 264],
                                start=(hf == 0), stop=(hf == 1))
                        dr = work_pool.tile([128, 8], f32, tag="dr")
                        nc.vector.tensor_scalar_add(dr, ap_[:, 256:264], 1e-6)
                        nc.vector.reciprocal(dr, dr)
                        at = work_pool.tile([128, 256], f16, tag="attn")
                        nc.vector.tensor_tensor(
                            out=at.rearrange("p (h e) -> p h e", e=32),
                            in0=ap_[:, 0:256].rearrange("p (h e) -> p h e", e=32),
                            in1=dr.rearrange("p (h u) -> p h u", u=1)
                                 .to_broadcast([128, 8, 32]),
                            op=mybir.AluOpType.mult)
                        aT = work_pool.tile([128, 256], f16, tag="aT")
                        tp = tpsum.tile([128, 256], f16, tag="tr")
                        for kd in range(2):
                            nc.tensor.transpose(
                                tp[:, kd * 128:(kd + 1) * 128],
                                at[:, kd * 128:(kd + 1) * 128], ident_h)
                        nc.scalar.copy(aT, tp)
                        w4 = n % 4
                        if w4 % 2 == 0:
                            pp = pj_psum.tile([128, 512], f32, tag="pj")
                        pc = (w4 % 2) * 256
                        nc.tensor.matmul(pp[:, pc:pc + 256], aT[:, 0:128],
                                         wp_sb[:, 0:256],
                                         start=True, stop=False)
                        nc.tensor.matmul(pp[:, pc:pc + 256], aT[:, 128:256],
                                         wp_sb[:, 256:512],
                                         start=False, stop=True)
                        if w4 == 0:
                            ob = ob_pool.tile([128, 4 * 256], f32, tag="ob")
                        if w4 % 2 == 1:
                            obs = ob[:, (w4 - 1) * 256:(w4 + 1) * 256]
                            if bp_zero:
                                # zero bias: PSUM drain is a plain ACT copy
                                nc.scalar.copy(obs, pp)
                            else:
                                nc.vector.tensor_tensor(
                                    out=obs, in0=pp, in1=bp_bc2,
                                    op=mybir.AluOpType.add)
                        if w4 == 3:
                            n0 = n - 3
                            dst = out_d[n0 * 128:(n0 + 4) * 128, :] \
                                .rearrange("(w p) c -> p w c", p=128)
                            nc.sync.dma_start(
                                dst, ob.rearrange("p (w c) -> p w c", c=256))

            if profile_single:
                attention_stage(idx_by_b[0])
            else:
                pid = nc.partition_id()
                with tc.If(pid <= 3) as cmp:
                    attention_stage(idx_by_b[0])
                with cmp.Else():
                    attention_stage(idx_by_b[1])

    nc.compile()
    return nc


def kernel(x, W_qkv, b_qkv, W_proj, b_proj):
    global last_results, last_nc, last_in_maps
    from concourse import bass_utils

    x = np.asarray(x, dtype=np.float32)
    xw = _windowize(x)                                     # [T,B,NW,WS,C]
    idx = _routing_idx(xw)                                 # [B,NW,TOPK]

    bq_zero = bool(np.all(np.asarray(b_qkv) == 0.0))
    bp_zero = bool(np.all(np.asarray(b_proj) == 0.0))
    nc = _build_program(idx, bq_zero=bq_zero, bp_zero=bp_zero)

    mask = np.zeros((128, 528), np.float32)
    for hf in range(2):
        for cr in range(128):
            h = hf * 4 + cr // 32                  # global head of row cr
            mask[cr, hf * 264 + h * 32:hf * 264 + (h + 1) * 32] = 1.0
            mask[cr, hf * 264 + 256 + h] = 1.0

    import ml_dtypes
    E4 = ml_dtypes.float8_e4m3
    wq32 = np.asarray(W_qkv, np.float32)
    wqh = wq32.astype(np.float16)
    wl8 = (-2048.0 * (wq32 - wqh.astype(np.float32))).astype(E4)
    wh8 = (-wqh.astype(np.float32)).astype(E4)
    in_maps = []
    for core in range(N_CORES):
        b, t = divmod(core, T)
        xwT_c = np.ascontiguousarray(xw[t, b].reshape(NTOK, C).T)  # [C,NTOK]
        xh = xwT_c.astype(np.float16)
        xh_s = (xh.astype(np.float32) * 2048.0).astype(np.float16)
        xw8_ = xh.astype(np.float32).astype(E4)
        xl8_ = (2048.0 * (xwT_c - xh.astype(np.float32))).astype(E4)
        in_maps.append({
            "xwT": xh_s,
            "xw8": xw8_,
            "xl8": xl8_,
            "masks": mask.astype(np.float16),
            "wq": wqh,
            "wl8": wl8,
            "wh8": wh8,
            "bq": np.asarray(b_qkv, np.float32),
            "wp": np.asarray(W_proj, np.float16),
            "bp": np.asarray(b_proj, np.float32),
        })

    res = bass_utils.run_bass_kernel_spmd(
        nc, in_maps, core_ids=list(range(N_CORES)), trace=False)
    last_results = res
    last_nc, last_in_maps = nc, in_maps

    ow = np.empty((T, B, NW, WS, C), np.float32)
    for core in range(N_CORES):
        b, t = divmod(core, T)
        ow[t, b] = res.results[core]["out"].reshape(NW, WS, C)
    return _unwindowize(ow)
